# revision 12
# baseline (speedup 1.0000x reference)
"""CenterLoss kernel for Trainium2 (raw Bass/Bacc, no Tile), 8-core
data-parallel.

Key algebraic insight: the reference builds the full [B, C] squared-
distance matrix and masks it with one-hot(labels), so only
distmat[i, labels[i]] survives.  The loss is therefore

    loss = (1/B) * sum_i || x_i - centers[labels[i]] ||^2

which needs only a gather of each sample's center row, not the
[4096, 10000] matmul.

Sharding: data-parallel over the batch.  Each of the 8 cores gets 512
samples (x shard + int16 gather indices) and the full replicated centers
table in DRAM; it gathers its 512 center rows, computes per-partition
partial sums of ||x - c||^2 on device, and the host reduces the 8x[128,2]
partials.

v3: the dominant cost in v1 was SWDGE descriptor generation on the GpSimd
Q7 core — 4 generic indirect gathers x ~1.4us apiece (the ~1us fixed
overhead per SWDGE instruction dwarfs the per-descriptor part).  v3 uses
the `dma_gather` custom Q7 instruction (mlp library) instead: 256 indices
per instruction, descriptor generation vectorized 16 lanes at a time with
the tx/rx streams built on two Q7 cores in parallel.  queue_num=0/1 place
the two halves on different Q7 core pairs so even the two instructions'
heavy work can overlap.  The mlp library load is issued first so it hides
under the index-DMA round trip.  The tail skips v1's PE partition-reduce
+ PSUM copy: Scalar's Square-activation accumulator columns go straight
to DRAM ([128,2] f32) and the host does the final 256-element sum.

dma_gather layout (blocked): gathered row j lands at partition j%128,
chunk j//128 of a [128, chunks, 512] SBUF tile; x is DMA'd to match
(sample j = x[j] on partition j%128, chunk j//128).  Indices are int16,
wrapped 16-wide (index j at partition j%16, column j//16) and replicated
to all 128 partitions because each Q7 core reads its own 16-partition
SBUF window.

Per core:
  Sync   : idx DMA ([128,32] int16), then x as two [128,2,512] DMAs
           (2 KB contiguous per-partition strips)
  GpSimd : mlp library load; 2 x dma_gather (256 rows, queues 0/1)
  Vector : per-half subtract (xt - ct)
  Scalar : per-half Square activation w/ accum -> partials[:,h]; final
           out DMA (HWDGE on ACT) of partials [128,2]
Host: sum(partials) / BATCH, summed over the 8 cores.

Manual semaphores; no Tile exit drain+butterfly+sem-clear (the bass entry
preamble clears sems, so re-execution stays safe).
"""

from contextlib import ExitStack

import numpy as np

import concourse.bacc as bacc
from concourse import library_config, mybir
from concourse.bass_utils import run_bass_kernel_spmd

BATCH = 4096
NUM_CLASSES = 10000
FEAT_DIM = 512
N_CORES = 8
BPC = BATCH // N_CORES   # samples per core = 512
P = 128                  # SBUF partitions
CHUNKS = BPC // P        # 4 chunks of 128 samples per core
HALF = CHUNKS // 2       # 2 chunks per gather half
IDX_COLS = BPC // 16     # idx tile free dim = 32

AF = mybir.AluOpType

_NC_CACHE = {}


def _build_bass():
    nc = bacc.Bacc(None, target_bir_lowering=False)

    x_in = nc.dram_tensor("x", [BPC, FEAT_DIM], mybir.dt.float32,
                          kind="ExternalInput")
    idx_in = nc.dram_tensor("idx", [P, IDX_COLS], mybir.dt.int16,
                            kind="ExternalInput")
    cen_in = nc.dram_tensor("centers", [NUM_CLASSES, FEAT_DIM],
                            mybir.dt.float32, kind="ExternalInput")
    out_t = nc.dram_tensor("out", [P, 2], mybir.dt.float32,
                           kind="ExternalOutput")

    with ExitStack() as ctx:
        ec = ctx.enter_context
        idx_sb = ec(nc.sbuf_tensor("idx_sb", [P, IDX_COLS], mybir.dt.int16))
        xt = ec(nc.sbuf_tensor("xt", [P, CHUNKS, FEAT_DIM],
                               mybir.dt.float32))
        ct = ec(nc.sbuf_tensor("ct", [P, CHUNKS, FEAT_DIM],
                               mybir.dt.float32))
        dd = ec(nc.sbuf_tensor("dd", [P, CHUNKS, FEAT_DIM],
                               mybir.dt.float32))
        sq = ec(nc.sbuf_tensor("sq", [P, CHUNKS, FEAT_DIM],
                               mybir.dt.float32))
        partials = ec(nc.sbuf_tensor("partials", [P, 2], mybir.dt.float32))
        s_idx = ec(nc.semaphore("s_idx"))
        s_xs = [ec(nc.semaphore(f"s_x{h}")) for h in range(2)]
        s_cts = [ec(nc.semaphore(f"s_ct{h}")) for h in range(2)]
        s_sub = ec(nc.semaphore("s_sub"))
        s_sq = ec(nc.semaphore("s_sq"))
        s_out = ec(nc.semaphore("s_out"))

        # ---- Sync: indices first (gathers depend on them), then x in two
        # halves, blocked layout: sample j -> partition j%128, chunk j//128
        # (2 KB contiguous strips; partition p, half h holds rows
        # 256h + 128g + p for g in {0,1}).
        nc.sync.dma_start(out=idx_sb[:], in_=idx_in[:]).then_inc(s_idx, 16)
        for h in range(2):
            nc.sync.dma_start(
                out=xt[:, 2 * h:2 * h + 2, :],
                in_=x_in[2 * h * P:(2 * h + 2) * P, :]
                .rearrange("(g p) f -> p g f", g=HALF),
            ).then_inc(s_xs[h], 16)

        # ---- GpSimd: library load (overlaps the idx DMA round trip), then
        # one vectorized dma_gather per half on separate Q7 core pairs ----
        nc.gpsimd.load_library(library_config.mlp)
        nc.gpsimd.wait_ge(s_idx, 16)
        for h in range(2):
            nc.gpsimd.dma_gather(
                out_ap=ct[:, 2 * h:2 * h + 2, :],
                in_ap=cen_in[:],
                idxs_ap=idx_sb[:, h * (IDX_COLS // 2):(h + 1) * (IDX_COLS // 2)],
                num_idxs=2 * P,
                num_idxs_reg=2 * P,
                elem_size=FEAT_DIM,
                queue_num=0,
            ).then_inc(s_cts[h], 16)

        # ---- Vector: per-half subtract ----
        for h in range(2):
            sl = slice(2 * h, 2 * h + 2)
            nc.vector.wait_ge(s_xs[h], 16)
            nc.vector.wait_ge(s_cts[h], 16)
            nc.vector.tensor_tensor(
                out=dd[:, sl, :], in0=xt[:, sl, :], in1=ct[:, sl, :],
                op=AF.subtract).then_inc(s_sub, 1)

        # ---- Scalar: per-half square + free-dim accumulate ----
        for h in range(2):
            sl = slice(2 * h, 2 * h + 2)
            nc.scalar.wait_ge(s_sub, h + 1)
            nc.scalar.activation(
                out=sq[:, sl, :], in_=dd[:, sl, :],
                func=mybir.ActivationFunctionType.Square,
                accum_out=partials[:, h:h + 1]).then_inc(s_sq, 1)

        # ---- Scalar: output DMA (HWDGE on ACT).  No completion wait: the
        # NRT exit barrier's Drain empties the HWDGE queue before execution
        # is reported complete. ----
        nc.scalar.wait_ge(s_sq, 2)
        nc.scalar.dma_start(out=out_t[:], in_=partials[:]).then_inc(s_out, 16)

    # Bacc defers register allocation + event-semaphore splitting to
    # compile(); the pjrt exec path serializes without calling it.
    nc.compile()
    return nc


def get_nc():
    if "nc" not in _NC_CACHE:
        _NC_CACHE["nc"] = _build_bass()
    return _NC_CACHE["nc"]


def _make_idx(labels_core):
    # index j at partition j%16, column j//16, replicated to 128 partitions
    block = labels_core.astype(np.int16).reshape(IDX_COLS, 16).T  # [16, 32]
    return np.ascontiguousarray(np.tile(block, (P // 16, 1)))     # [128, 32]


def kernel(x, labels, centers, _run_kwargs=None):
    x = np.ascontiguousarray(x, dtype=np.float32)
    labels = np.ascontiguousarray(labels).astype(np.int32)
    centers = np.ascontiguousarray(centers, dtype=np.float32)

    nc = get_nc()
    in_maps = [
        {
            "x": x[c * BPC:(c + 1) * BPC],
            "idx": _make_idx(labels[c * BPC:(c + 1) * BPC]),
            "centers": centers,
        }
        for c in range(N_CORES)
    ]
    kwargs = _run_kwargs or {}
    out = run_bass_kernel_spmd(nc, in_maps, core_ids=list(range(N_CORES)),
                               **kwargs)
    # reduce the 8 per-core [128, 2] partial-sum tiles on the host
    total = np.float64(0.0)
    for r in out.results:
        total += np.asarray(r["out"], dtype=np.float64).sum()
    if kwargs:
        kernel.last_run = out
    return np.asarray(total / BATCH, dtype=np.float32)


# revision 13
# speedup vs baseline: 1.4344x; 1.4344x over previous
"""CenterLoss kernel for Trainium2 (raw Bass/Bacc, no Tile), 8-core
data-parallel.

Key algebraic insight: the reference builds the full [B, C] squared-
distance matrix and masks it with one-hot(labels), so only
distmat[i, labels[i]] survives.  The loss is therefore

    loss = (1/B) * sum_i || x_i - centers[labels[i]] ||^2

which needs only a gather of each sample's center row (indirect DMA), not
the [4096, 10000] matmul.

Sharding: data-parallel over the batch.  Each of the 8 cores gets 512
samples (x shard + labels shard) and the full replicated centers table in
DRAM; it gathers its 512 center rows, computes per-partition partial sums
of ||x - c||^2 on device, and the host reduces the 8x[128,4] partials.

v4 vs v1 (21997 ns): same gather structure (4 single-index-per-partition
SWDGE gathers — the HW ucode only supports one index per partition per
instruction; multi-column offset APs gather garbage, and the dma_gather
custom op costs an ~11us mlp library load per execution).  The tail is
restructured: v1's PE partition-reduce + PSUM->SBUF copy + Sync handoff
are dropped — per-partition accumulator columns go straight to DRAM
([128,4] f32) from the Scalar engine and the host does the final
512-element sum.  Scalar's ACTIVATE+READ_ACCUMULATOR pitch (~1us/chunk)
made it the tail pacer, so chunk 3 (the last to land) is squared+reduced
on the Vector engine (tensor_tensor mult + tensor_reduce add) instead,
letting its reduction overlap Scalar's chunk-2 work.

Per core (512 samples = 4 chunks x 128 partitions, interleaved layout:
chunk a holds samples {4p + a}, one per partition p):
  Sync   : labels DMA ([128,4] int32 tile), then x as two DMAs with 4 KB
           contiguous per-partition strips
  GpSimd : 4 indirect gathers (offset AP = labels column a)
  Vector : per-chunk subtract; chunk 3 also mult+reduce -> partials[:,3]
  Scalar : chunks 0-2 Square activation w/ accum -> partials[:,a]; final
           out DMA (HWDGE on ACT) of partials [128,4]
Host: sum(partials) / BATCH, summed over the 8 cores.

Manual semaphores; no Tile exit drain+butterfly+sem-clear (the bass entry
preamble clears sems, so re-execution stays safe).
"""

from contextlib import ExitStack

import numpy as np

import concourse.bacc as bacc
import concourse.bass as bass
from concourse import mybir
from concourse.bass_utils import run_bass_kernel_spmd

BATCH = 4096
NUM_CLASSES = 10000
FEAT_DIM = 512
N_CORES = 8
BPC = BATCH // N_CORES   # samples per core = 512
P = 128                  # SBUF partitions
CHUNKS = BPC // P        # 4 chunks of 128 samples per core

AF = mybir.AluOpType

_NC_CACHE = {}


def _build_bass():
    nc = bacc.Bacc(None, target_bir_lowering=False)

    x_in = nc.dram_tensor("x", [BPC, FEAT_DIM], mybir.dt.float32,
                          kind="ExternalInput")
    lab_in = nc.dram_tensor("labels", [BPC], mybir.dt.int32,
                            kind="ExternalInput")
    cen_in = nc.dram_tensor("centers", [NUM_CLASSES, FEAT_DIM],
                            mybir.dt.float32, kind="ExternalInput")
    out_t = nc.dram_tensor("out", [P, CHUNKS], mybir.dt.float32,
                           kind="ExternalOutput")

    with ExitStack() as ctx:
        ec = ctx.enter_context
        lab_sb = ec(nc.sbuf_tensor("lab_sb", [P, CHUNKS], mybir.dt.int32))
        xt = ec(nc.sbuf_tensor("xt", [P, CHUNKS * FEAT_DIM],
                               mybir.dt.float32))
        ct = ec(nc.sbuf_tensor("ct", [P, CHUNKS * FEAT_DIM],
                               mybir.dt.float32))
        dds = [ec(nc.sbuf_tensor(f"dd{a}", [P, FEAT_DIM], mybir.dt.float32))
               for a in range(CHUNKS)]
        sqs = [ec(nc.sbuf_tensor(f"sq{a}", [P, FEAT_DIM], mybir.dt.float32))
               for a in range(CHUNKS)]
        partials = ec(nc.sbuf_tensor("partials", [P, CHUNKS],
                                     mybir.dt.float32))
        s_lab = ec(nc.semaphore("s_lab"))
        s_xs = [ec(nc.semaphore(f"s_x{h}")) for h in range(2)]
        s_cts = [ec(nc.semaphore(f"s_ct{a}")) for a in range(CHUNKS)]
        s_sub = ec(nc.semaphore("s_sub"))
        s_sq = ec(nc.semaphore("s_sq"))
        s_v3 = ec(nc.semaphore("s_v3"))
        s_out = ec(nc.semaphore("s_out"))

        # ---- Sync: labels first (gathers depend on them), then x as two
        # halves with 4 KB contiguous per-partition strips (partition p holds
        # rows 4p..4p+3; half h covers chunks {2h, 2h+1} = rows 4p+2h, 4p+2h+1).
        nc.sync.dma_start(
            out=lab_sb[:],
            in_=lab_in[:].rearrange("(p a) -> p a", a=CHUNKS),
        ).then_inc(s_lab, 16)
        H = CHUNKS // 2
        for h in range(2):
            nc.sync.dma_start(
                out=xt[:, h * H * FEAT_DIM:(h + 1) * H * FEAT_DIM],
                in_=x_in[:].rearrange(
                    "(p h g) f -> p h (g f)", h=2, g=H)[:, h, :],
            ).then_inc(s_xs[h], 16)

        # ---- GpSimd: 4 indirect gathers (SWDGE), one index per partition
        # per instruction (the only HW-supported indirect form) ----
        nc.gpsimd.wait_ge(s_lab, 16)
        for a in range(CHUNKS):
            nc.gpsimd.indirect_dma_start(
                out=ct[:, a * FEAT_DIM:(a + 1) * FEAT_DIM],
                out_offset=None,
                in_=cen_in[:],
                in_offset=bass.IndirectOffsetOnAxis(
                    ap=lab_sb[:, a:a + 1], axis=0),
            ).then_inc(s_cts[a], 16)

        # ---- Vector: per-chunk subtract; chunk 3 (last to land) also gets
        # its square + free-dim reduce here so it overlaps Scalar's chunk-2
        # ACTIVATE+READ_ACCUMULATOR (~1us/chunk, the tail pacer) ----
        for a in range(CHUNKS):
            sl = slice(a * FEAT_DIM, (a + 1) * FEAT_DIM)
            nc.vector.wait_ge(s_xs[a // (CHUNKS // 2)], 16)
            nc.vector.wait_ge(s_cts[a], 16)
            nc.vector.tensor_tensor(
                out=dds[a][:], in0=xt[:, sl], in1=ct[:, sl],
                op=AF.subtract).then_inc(s_sub, 1)
        a3 = CHUNKS - 1
        nc.vector.wait_ge(s_sub, CHUNKS)
        nc.vector.tensor_tensor(
            out=sqs[a3][:], in0=dds[a3][:], in1=dds[a3][:],
            op=AF.mult).then_inc(s_v3, 1)
        nc.vector.wait_ge(s_v3, 1)
        nc.vector.tensor_reduce(
            out=partials[:, a3:a3 + 1], in_=sqs[a3][:],
            axis=mybir.AxisListType.X, op=AF.add).then_inc(s_v3, 1)

        # ---- Scalar: chunks 0-2 square + free-dim accumulate ----
        for a in range(CHUNKS - 1):
            nc.scalar.wait_ge(s_sub, a + 1)
            nc.scalar.activation(
                out=sqs[a][:], in_=dds[a][:],
                func=mybir.ActivationFunctionType.Square,
                accum_out=partials[:, a:a + 1]).then_inc(s_sq, 1)

        # ---- Scalar: output DMA (HWDGE on ACT).  No completion wait: the
        # NRT exit barrier's Drain empties the HWDGE queue before execution
        # is reported complete. ----
        nc.scalar.wait_ge(s_sq, CHUNKS - 1)
        nc.scalar.wait_ge(s_v3, 2)
        nc.scalar.dma_start(out=out_t[:], in_=partials[:]).then_inc(s_out, 16)

    # Bacc defers register allocation + event-semaphore splitting to
    # compile(); the pjrt exec path serializes without calling it.
    nc.compile()
    return nc


def get_nc():
    if "nc" not in _NC_CACHE:
        _NC_CACHE["nc"] = _build_bass()
    return _NC_CACHE["nc"]


def kernel(x, labels, centers, _run_kwargs=None):
    x = np.ascontiguousarray(x, dtype=np.float32)
    labels = np.ascontiguousarray(labels).astype(np.int32)
    centers = np.ascontiguousarray(centers, dtype=np.float32)

    nc = get_nc()
    in_maps = [
        {
            "x": x[c * BPC:(c + 1) * BPC],
            "labels": labels[c * BPC:(c + 1) * BPC],
            "centers": centers,
        }
        for c in range(N_CORES)
    ]
    kwargs = _run_kwargs or {}
    out = run_bass_kernel_spmd(nc, in_maps, core_ids=list(range(N_CORES)),
                               **kwargs)
    # reduce the 8 per-core [128, 4] partial-sum tiles on the host
    total = np.float64(0.0)
    for r in out.results:
        total += np.asarray(r["out"], dtype=np.float64).sum()
    if kwargs:
        kernel.last_run = out
    return np.asarray(total / BATCH, dtype=np.float32)
